# revision 12
# baseline (speedup 1.0000x reference)
"""Trainium2 Bass kernel for nn_DeepGraph (5-layer GAT + layernorms + per-node heads).

Strategy (8 NeuronCores, SPMD):
  - dst-shard: core c owns nodes [c*4096, (c+1)*4096) (= head groups 8c..8c+8).
  - Within a core, dst nodes are sorted by in-degree and grouped into 32 blocks
    of 128; messages for a block are gathered dst-major (msg i = k*128 + j ->
    partition j = dst j, chunk k = edge slot) via gpsimd dma_gather from a
    DRAM node-feature table with 512B rows [h_bf16(128) | a_s_f32 | lo_bf16(126)].
  - Attention: e = leakyrelu(a_s[src] + a_d[dst]) with a_s riding the gathered
    row and a_d per-partition; softmax max-subtraction is skipped (e is small,
    mathematically invariant).
  - Segment sum: messages scaled by w (broadcast tensor_tensor), then
    identity-weight matmuls accumulate chunks in PSUM.
  - Per layer, each core computes LN+dense for its shard and the 512B rows are
    exchanged with an 8-rank AllGather into a Shared DRAM table.
  - Final head phase un-permutes rows with a second dma_gather, then runs the
    per-group MLPs + softmax on PE/ACT/DVE.
"""
import math
import os
import numpy as np
import ml_dtypes

import concourse.bass as bass
import concourse.bacc as bacc
import concourse.tile as tile
import concourse.mybir as mybir
from concourse.alu_op_type import AluOpType
from concourse import library_config
from concourse.bass_utils import run_bass_kernel_spmd

F32 = mybir.dt.float32
BF16 = mybir.dt.bfloat16
I16 = mybir.dt.int16
BF = ml_dtypes.bfloat16
import bass_rust
AXX = bass_rust.AxisListType.X

IN_C, HID, L, NN, BS, OUT = 32, 128, 4, 64, 512, 3
N = NN * BS                 # 32768
E = 524288
NCORE = 8
SHARD = N // NCORE          # 4096
NBLK = SHARD // 128         # 32
EPS_LN = 1e-5
ROWS = 256                  # bf16 slots per table row (512B)
LOCH = 126                  # lo-correction channels stored (slots 130:256)


# ---------------------------------------------------------------- host prep --

def _host_prep(x, edge_index, W0, att_src0, att_dst0):
    src = np.asarray(edge_index[0], dtype=np.int64)
    dst = np.asarray(edge_index[1], dtype=np.int64)
    loops = np.arange(N, dtype=np.int64)
    src_all = np.concatenate([src, loops])
    dst_all = np.concatenate([dst, loops])

    counts = np.bincount(dst_all, minlength=N).astype(np.int64)
    order = np.argsort(dst_all, kind="stable")
    src_sorted = src_all[order]
    starts = np.zeros(N + 1, dtype=np.int64)
    np.cumsum(counts, out=starts[1:])

    # per-core degree-sorted permutation
    perms = []
    for c in range(NCORE):
        lo = c * SHARD
        p = np.argsort(counts[lo:lo + SHARD], kind="stable") + lo
        perms.append(p)
    gperm = np.concatenate(perms)              # global permuted order -> natural
    ginv = np.empty(N, dtype=np.int64)
    ginv[gperm] = np.arange(N)

    # uniform per-block degree schedule (max across cores)
    D_sched = np.zeros(NBLK, dtype=np.int64)
    for b in range(NBLK):
        for c in range(NCORE):
            D_sched[b] = max(D_sched[b], counts[perms[c][(b + 1) * 128 - 1]])
    DTOT = int(D_sched.sum())

    # per-core idx tables and masks
    idx_wrapped_all = []
    mask_all = []
    for c in range(NCORE):
        idx_core = np.zeros((DTOT, 128), dtype=np.int64)
        mask_core = np.zeros((DTOT, 128), dtype=np.float32)
        off = 0
        for b in range(NBLK):
            Db = int(D_sched[b])
            for j in range(128):
                n = perms[c][b * 128 + j]
                cnt = int(counts[n])
                vals = ginv[src_sorted[starts[n]:starts[n] + cnt]]
                col = idx_core[off:off + Db, j]
                col[:cnt] = vals
                col[cnt:] = ginv[n]
                mask_core[off:off + cnt, j] = 1.0
            off += Db
        flat = idx_core.reshape(-1)            # i = k*128 + j within blocks
        assert flat.max() < 32768 and flat.min() >= 0
        ia = flat.astype(np.int16).reshape(-1, 16)
        wrapped = np.tile(ia.T, (8, 1))        # [128, DTOT*128/16]
        idx_wrapped_all.append(np.ascontiguousarray(wrapped))
        mask_all.append(np.ascontiguousarray(mask_core.T))   # [128, DTOT]

    # layer-0 table (permuted row order)
    hg0 = x.astype(np.float32) @ W0.astype(np.float32)       # [N, 128]
    a_s0 = hg0 @ att_src0.astype(np.float32)
    a_d0 = hg0 @ att_dst0.astype(np.float32)
    g0 = np.zeros((N, ROWS), dtype=BF)
    hp = hg0[gperm]
    g0[:, 0:HID] = hp.astype(BF)
    lo = (hp - g0[:, 0:HID].astype(np.float32))[:, :LOCH]
    g0[:, 130:130 + LOCH] = lo.astype(BF)
    g0.view(np.uint16)[:, 128:130] = (
        a_s0[gperm].astype("<f4")[:, None].view("<u2").reshape(-1, 2))

    # a_d0 per core in [128, NBLK] layout
    a_d0_tiles = []
    inv_local = []
    for c in range(NCORE):
        t = a_d0[perms[c]].reshape(NBLK, 128).T.astype(np.float32)
        a_d0_tiles.append(np.ascontiguousarray(t))
        il = np.empty(SHARD, dtype=np.int64)
        il[perms[c] - c * SHARD] = np.arange(SHARD)
        ia = il.astype(np.int16).reshape(-1, 16)
        inv_local.append(np.ascontiguousarray(np.tile(ia.T, (8, 1))))

    return dict(D_sched=D_sched, DTOT=DTOT, idx=idx_wrapped_all, mask=mask_all,
                g0=g0, a_d0=a_d0_tiles, inv_local=inv_local, perms=perms,
                gperm=gperm, counts=counts)


# --------------------------------------------------------------- bass kernel --

def _build_nc(D_sched, DTOT, phase="full"):
    nc = bacc.Bacc("TRN2", target_bir_lowering=False)

    g0_d = nc.dram_tensor("g0", [N, ROWS], BF16, kind="ExternalInput")
    idx_d = nc.dram_tensor("idx", [128, DTOT * 8], I16, kind="ExternalInput")
    mask_d = nc.dram_tensor("mask", [128, DTOT], F32, kind="ExternalInput")
    ad0_d = nc.dram_tensor("ad0", [128, NBLK], F32, kind="ExternalInput")
    invl_d = nc.dram_tensor("invl", [128, SHARD // 16], I16, kind="ExternalInput")
    wcat_d = nc.dram_tensor("wcat", [128, L, 132], F32, kind="ExternalInput")
    gam_d = nc.dram_tensor("gam", [128, L], F32, kind="ExternalInput")
    bet_d = nc.dram_tensor("bet", [128, L], F32, kind="ExternalInput")
    biasr_d = nc.dram_tensor("biasr", [128, L + 1, HID], F32, kind="ExternalInput")
    g0rep_d = nc.dram_tensor("g0rep", [128, HID], F32, kind="ExternalInput")
    b0rep_d = nc.dram_tensor("b0rep", [128, HID], F32, kind="ExternalInput")
    idf_d = nc.dram_tensor("idf", [128, 128], F32, kind="ExternalInput")
    idb_d = nc.dram_tensor("idb", [128, 128], BF16, kind="ExternalInput")
    w1p_d = nc.dram_tensor("w1p", [128, 8 * 64], F32, kind="ExternalInput")
    w2p_d = nc.dram_tensor("w2p", [64, 8 * OUT], F32, kind="ExternalInput")

    out_d = nc.dram_tensor("out", [SHARD, OUT], F32, kind="ExternalOutput")
    outh_d = nc.dram_tensor("outh", [SHARD, HID], F32, kind="ExternalOutput")

    CSL = 132                   # compact row slots exchanged (hi + a_s fp32)
    cc_in = nc.dram_tensor("cc_in", [SHARD, CSL], BF16, kind="Internal")
    gtab_sm = nc.dram_tensor("gtab_sm", [N, CSL], BF16, kind="Internal",
                             addr_space="Shared")
    gtab = nc.dram_tensor("gtab", [N, ROWS], BF16, kind="Internal")
    fin_d = nc.dram_tensor("fin", [SHARD, HID], F32, kind="Internal")

    DMAX = int(max(D_sched))
    offs = np.concatenate([[0], np.cumsum(D_sched)]).astype(int)

    with tile.TileContext(nc) as tc:
        with (
            tc.tile_pool(name="persist", bufs=1) as pp,
            tc.tile_pool(name="mg", bufs=3) as mgp,
            tc.tile_pool(name="small", bufs=4) as sp,
            tc.tile_pool(name="dense", bufs=3) as dp,
            tc.tile_pool(name="psacc", bufs=2, space="PSUM") as ps_acc,
            tc.tile_pool(name="psmm", bufs=2, space="PSUM") as ps_mm,
            tc.tile_pool(name="pstp", bufs=2, space="PSUM") as psb,
            tc.tile_pool(name="psh", bufs=1, space="PSUM") as ps_h,
            tc.tile_pool(name="psl", bufs=1, space="PSUM") as ps_l,
        ):
            nc.gpsimd.load_library(library_config.mlp)

            # ---- persistent loads
            idx_t = pp.tile([128, DTOT * 8], I16)
            nc.sync.dma_start(idx_t[:], idx_d[:])
            mask_t = pp.tile([128, DTOT], F32)
            nc.sync.dma_start(mask_t[:], mask_d[:])
            ad0_t = pp.tile([128, NBLK], F32)
            nc.sync.dma_start(ad0_t[:], ad0_d[:])
            invl_t = pp.tile([128, SHARD // 16], I16)
            nc.sync.dma_start(invl_t[:], invl_d[:])
            wcat_t = pp.tile([128, L, 132], F32)
            nc.sync.dma_start(wcat_t[:], wcat_d[:])
            gam_t = pp.tile([128, L], F32)
            nc.sync.dma_start(gam_t[:], gam_d[:])
            bet_t = pp.tile([128, L], F32)
            nc.sync.dma_start(bet_t[:], bet_d[:])
            biasr_t = pp.tile([128, L + 1, HID], F32)
            nc.sync.dma_start(biasr_t[:], biasr_d[:])
            g0rep_t = pp.tile([128, HID], F32)
            nc.sync.dma_start(g0rep_t[:], g0rep_d[:])
            b0rep_t = pp.tile([128, HID], F32)
            nc.sync.dma_start(b0rep_t[:], b0rep_d[:])
            idf_t = pp.tile([128, 128], F32)
            nc.sync.dma_start(idf_t[:], idf_d[:])
            idb_t = pp.tile([128, 128], BF16)
            nc.sync.dma_start(idb_t[:], idb_d[:])
            w1p_t = pp.tile([128, 8 * 64], F32)
            nc.sync.dma_start(w1p_t[:], w1p_d[:])
            w2p_t = pp.tile([64, 8 * OUT], F32)
            nc.sync.dma_start(w2p_t[:], w2p_d[:])

            h_t = pp.tile([128, NBLK, HID], F32)       # node state (permuted)
            asd_t = pp.tile([128, NBLK, 2], F32)       # per-layer a_s/a_d own
            grow_t = pp.tile([128, NBLK, CSL], BF16)   # assembled rows (compact)

            # ---------------- aggregation over one table ----------------
            def aggregate(table_ap, layer):
                """layer 0: h = relu(agg + b0); layer>=1: h += agg + b_res."""
                for b in range(NBLK):
                    Db = int(D_sched[b])
                    mg = mgp.tile([128, DMAX, ROWS], BF16, tag="mg")
                    i0 = offs[b] * 8
                    nc.gpsimd.dma_gather(
                        mg[:, 0:Db, :], table_ap, idx_t[:, i0:i0 + Db * 8],
                        128 * Db, 128 * Db, ROWS, single_packet=False)

                    mg32 = mg[:, 0:Db, :].bitcast(F32)   # [128, Db, 128]
                    a_s = mg32[:, :, 64]                 # strided [128, Db]
                    if layer == 0:
                        a_d = ad0_t[:, b:b + 1]
                    else:
                        a_d = asd_t[:, b, 1:2]
                    s_t = sp.tile([128, DMAX], F32, tag="s")
                    nc.vector.tensor_scalar(s_t[:, 0:Db], a_s, a_d, None,
                                            AluOpType.add)
                    s2_t = sp.tile([128, DMAX], F32, tag="s2")
                    nc.vector.tensor_scalar(s2_t[:, 0:Db], s_t[:, 0:Db], 0.2,
                                            None, AluOpType.mult)
                    e_t = sp.tile([128, DMAX], F32, tag="e")
                    nc.vector.tensor_tensor(e_t[:, 0:Db], s_t[:, 0:Db],
                                            s2_t[:, 0:Db], AluOpType.max)
                    x_t = sp.tile([128, DMAX], F32, tag="x")
                    nc.scalar.activation(x_t[:, 0:Db], e_t[:, 0:Db],
                                         mybir.ActivationFunctionType.Exp)
                    wm_t = sp.tile([128, DMAX], F32, tag="wm")
                    nc.vector.tensor_tensor(wm_t[:, 0:Db], x_t[:, 0:Db],
                                            mask_t[:, offs[b]:offs[b] + Db],
                                            AluOpType.mult)
                    z_t = sp.tile([128, 1], F32, tag="z")
                    nc.vector.tensor_reduce(z_t[:], wm_t[:, 0:Db], AXX,
                                            AluOpType.add)
                    # reproduce the oracle's segment_max->segment_sum lowering:
                    # alpha = w / (z + 1e-16*exp(S)), S = sum of masked logits
                    em_t = sp.tile([128, DMAX], F32, tag="em")
                    nc.vector.tensor_tensor(em_t[:, 0:Db], e_t[:, 0:Db],
                                            mask_t[:, offs[b]:offs[b] + Db],
                                            AluOpType.mult)
                    sS_t = sp.tile([128, 1], F32, tag="sS")
                    nc.vector.tensor_reduce(sS_t[:], em_t[:, 0:Db], AXX,
                                            AluOpType.add)
                    sB_t = sp.tile([128, 1], F32, tag="sB")
                    nc.vector.tensor_scalar(sB_t[:], sS_t[:], -36.841361487904734,
                                            None, AluOpType.add)
                    sE_t = sp.tile([128, 1], F32, tag="sE")
                    nc.scalar.activation(sE_t[:], sB_t[:],
                                         mybir.ActivationFunctionType.Exp)
                    z2_t = sp.tile([128, 1], F32, tag="z2")
                    nc.vector.tensor_tensor(z2_t[:], z_t[:], sE_t[:],
                                            AluOpType.add)
                    rz_t = sp.tile([128, 1], F32, tag="rz")
                    nc.vector.reciprocal(rz_t[:], z2_t[:])

                    mgw = mgp.tile([128, DMAX, HID], BF16, tag="mgw")
                    wview = wm_t[:, 0:Db, None].broadcast_to([128, Db, HID])
                    nc.vector.tensor_tensor(mgw[:, 0:Db, :], mg[:, 0:Db, 0:HID],
                                            wview, AluOpType.mult)

                    acc = ps_acc.tile([128, HID], F32, tag="acc")
                    for k in range(Db):
                        nc.tensor.matmul(acc[:], idb_t[:], mgw[:, k, :],
                                         start=(k == 0), stop=(k == Db - 1))

                    agg = sp.tile([128, HID], F32, tag="agg")
                    nc.vector.tensor_scalar(agg[:], acc[:], rz_t[:], None,
                                            AluOpType.mult)
                    if layer == 0:
                        t1 = sp.tile([128, HID], F32, tag="t1")
                        nc.vector.tensor_tensor(t1[:], agg[:],
                                                biasr_t[:, 0, :], AluOpType.add)
                        nc.vector.tensor_scalar(h_t[:, b, :], t1[:], 0.0, None,
                                                AluOpType.max)
                    else:
                        t1 = sp.tile([128, HID], F32, tag="t1")
                        nc.vector.tensor_tensor(t1[:], agg[:],
                                                biasr_t[:, layer, :],
                                                AluOpType.add)
                        nc.vector.tensor_tensor(h_t[:, b, :], h_t[:, b, :],
                                                t1[:], AluOpType.add)

            # ---------------- dense phase for layer i (1..L) ----------------
            def dense(layer):
                li = layer - 1
                mu_t = sp.tile([128, NBLK], F32, tag="mu")
                sq_t = sp.tile([128, NBLK], F32, tag="sq")
                scr = sp.tile([128, HID], F32, tag="scr")
                for c in range(NBLK):
                    nc.scalar.activation(scr[:], h_t[:, c, :],
                                         mybir.ActivationFunctionType.Copy,
                                         accum_out=mu_t[:, c:c + 1])
                    nc.scalar.activation(scr[:], h_t[:, c, :],
                                         mybir.ActivationFunctionType.Square,
                                         accum_out=sq_t[:, c:c + 1])
                m_t = sp.tile([128, NBLK], F32, tag="m")
                nc.vector.tensor_scalar(m_t[:], mu_t[:], 1.0 / HID, None,
                                        AluOpType.mult)
                ex2_t = sp.tile([128, NBLK], F32, tag="ex2")
                nc.vector.tensor_scalar(ex2_t[:], sq_t[:], 1.0 / HID, None,
                                        AluOpType.mult)
                msq_t = sp.tile([128, NBLK], F32, tag="msq")
                nc.vector.tensor_tensor(msq_t[:], m_t[:], m_t[:], AluOpType.mult)
                var_t = sp.tile([128, NBLK], F32, tag="var")
                nc.vector.tensor_tensor(var_t[:], ex2_t[:], msq_t[:],
                                        AluOpType.subtract)
                ve_t = sp.tile([128, NBLK], F32, tag="ve")
                nc.vector.tensor_scalar(ve_t[:], var_t[:], EPS_LN, None,
                                        AluOpType.add)
                sd_t = sp.tile([128, NBLK], F32, tag="sd")
                nc.scalar.activation(sd_t[:], ve_t[:],
                                     mybir.ActivationFunctionType.Sqrt)
                rs_t = sp.tile([128, NBLK], F32, tag="rs")
                nc.vector.reciprocal(rs_t[:], sd_t[:])
                for c in range(NBLK):
                    n1 = dp.tile([128, HID], F32, tag="n1")
                    nc.vector.tensor_scalar(n1[:], h_t[:, c, :], m_t[:, c:c + 1],
                                            rs_t[:, c:c + 1], AluOpType.subtract,
                                            AluOpType.mult)
                    tp = psb.tile([128, 128], F32, tag="tp")
                    nc.tensor.transpose(tp[:], n1[:], idf_t[:])
                    tT = dp.tile([128, HID], F32, tag="tT")
                    nc.scalar.activation(tT[:], tp[:],
                                         mybir.ActivationFunctionType.Relu,
                                         bias=bet_t[:, li:li + 1],
                                         scale=gam_t[:, li:li + 1])
                    mmo = ps_mm.tile([128, 132], F32, tag="mmo")
                    nc.tensor.matmul(mmo[:], tT[:], wcat_t[:, li, :],
                                     start=True, stop=True)
                    nc.vector.tensor_copy(grow_t[:, c, 0:HID],
                                          mmo[:, 0:HID])
                    grow32 = grow_t[:, c, :].bitcast(F32)
                    nc.vector.tensor_copy(grow32[:, 64:65], mmo[:, 128:129])
                    nc.vector.tensor_copy(asd_t[:, c, :], mmo[:, 128:130])
                nc.sync.dma_start(
                    cc_in[:].rearrange("(c p) s -> p c s", p=128), grow_t[:])
                nc.gpsimd.collective_compute(
                    "AllGather", AluOpType.bypass,
                    replica_groups=[list(range(NCORE))],
                    ins=[cc_in[:]], outs=[gtab_sm[:]])
                nc.sync.dma_start(gtab[:, 0:CSL], gtab_sm[:])

            # ---------------- final LN + heads ----------------
            def final_phase():
                mu_t = sp.tile([128, NBLK], F32, tag="mu")
                sq_t = sp.tile([128, NBLK], F32, tag="sq")
                scr = sp.tile([128, HID], F32, tag="scr")
                for c in range(NBLK):
                    nc.scalar.activation(scr[:], h_t[:, c, :],
                                         mybir.ActivationFunctionType.Copy,
                                         accum_out=mu_t[:, c:c + 1])
                    nc.scalar.activation(scr[:], h_t[:, c, :],
                                         mybir.ActivationFunctionType.Square,
                                         accum_out=sq_t[:, c:c + 1])
                m_t = sp.tile([128, NBLK], F32, tag="m")
                nc.vector.tensor_scalar(m_t[:], mu_t[:], 1.0 / HID, None,
                                        AluOpType.mult)
                ex2_t = sp.tile([128, NBLK], F32, tag="ex2")
                nc.vector.tensor_scalar(ex2_t[:], sq_t[:], 1.0 / HID, None,
                                        AluOpType.mult)
                msq_t = sp.tile([128, NBLK], F32, tag="msq")
                nc.vector.tensor_tensor(msq_t[:], m_t[:], m_t[:], AluOpType.mult)
                var_t = sp.tile([128, NBLK], F32, tag="var")
                nc.vector.tensor_tensor(var_t[:], ex2_t[:], msq_t[:],
                                        AluOpType.subtract)
                ve_t = sp.tile([128, NBLK], F32, tag="ve")
                nc.vector.tensor_scalar(ve_t[:], var_t[:], EPS_LN, None,
                                        AluOpType.add)
                sd_t = sp.tile([128, NBLK], F32, tag="sd")
                nc.scalar.activation(sd_t[:], ve_t[:],
                                     mybir.ActivationFunctionType.Sqrt)
                rs_t = sp.tile([128, NBLK], F32, tag="rs")
                nc.vector.reciprocal(rs_t[:], sd_t[:])
                finrows = pp.tile([128, NBLK, HID], F32)
                for c in range(NBLK):
                    n1 = dp.tile([128, HID], F32, tag="n1")
                    nc.vector.tensor_scalar(n1[:], h_t[:, c, :], m_t[:, c:c + 1],
                                            rs_t[:, c:c + 1], AluOpType.subtract,
                                            AluOpType.mult)
                    n2 = dp.tile([128, HID], F32, tag="n2")
                    nc.vector.tensor_tensor(n2[:], n1[:], g0rep_t[:],
                                            AluOpType.mult)
                    n3 = dp.tile([128, HID], F32, tag="n3")
                    nc.vector.tensor_tensor(n3[:], n2[:], b0rep_t[:],
                                            AluOpType.add)
                    nc.vector.tensor_scalar(finrows[:, c, :], n3[:], 0.0, None,
                                            AluOpType.max)
                nc.sync.dma_start(
                    fin_d[:].rearrange("(c p) d -> p c d", p=128), finrows[:])

                fnat = pp.tile([128, NBLK, HID], F32)
                nc.gpsimd.dma_gather(fnat[:], fin_d[:], invl_t[:],
                                     SHARD, SHARD, HID, single_packet=False)

                finT = pp.tile([128, NBLK * 128], F32)
                for c in range(NBLK):
                    tp = psb.tile([128, 128], F32, tag="tp")
                    nc.tensor.transpose(tp[:], fnat[:, c, :], idf_t[:])
                    nc.vector.tensor_copy(finT[:, c * 128:(c + 1) * 128], tp[:])

                out_sb = pp.tile([128, NBLK, OUT], F32)
                for g in range(8):
                    hps = ps_h.tile([64, 512], F32, tag="hps")
                    nc.tensor.matmul(hps[:], w1p_t[:, g * 64:(g + 1) * 64],
                                     finT[:, g * 512:(g + 1) * 512],
                                     start=True, stop=True)
                    t1T = dp.tile([64, 512], F32, tag="t1T")
                    nc.vector.tensor_scalar(t1T[:], hps[:], 0.0, None,
                                            AluOpType.max)
                    for q in range(4):
                        lps = ps_l.tile([128, OUT], F32, tag="lps")
                        nc.tensor.matmul(lps[:], t1T[:, q * 128:(q + 1) * 128],
                                         w2p_t[:, g * OUT:(g + 1) * OUT],
                                         start=True, stop=True)
                        cc = g * 4 + q
                        exl = dp.tile([128, OUT], F32, tag="exl")
                        nc.scalar.activation(exl[:], lps[:],
                                             mybir.ActivationFunctionType.Exp)
                        sml = dp.tile([128, 1], F32, tag="sml")
                        nc.vector.tensor_reduce(sml[:], exl[:], AXX,
                                                AluOpType.add)
                        rml = dp.tile([128, 1], F32, tag="rml")
                        nc.vector.reciprocal(rml[:], sml[:])
                        nc.vector.tensor_scalar(out_sb[:, cc, :], exl[:],
                                                rml[:], None, AluOpType.mult)
                nc.sync.dma_start(
                    out_d[:].rearrange("(c p) d -> p c d", p=128), out_sb[:])

            # ---------------- main program ----------------
            nlayer = {"l0": 0, "l1": 1, "full": L}.get(phase, L)
            aggregate(g0_d[:], 0)
            for layer in range(1, nlayer + 1):
                dense(layer)
                aggregate(gtab[:], layer)
            if phase == "full":
                final_phase()
            else:
                nc.sync.dma_start(
                    outh_d[:].rearrange("(c p) d -> p c d", p=128), h_t[:])

    nc.compile()
    return nc


_NC_CACHE = {}


def kernel(**inputs) -> np.ndarray:
    x = np.asarray(inputs["x"], dtype=np.float32)
    edge_index = np.asarray(inputs["edge_index"])
    W0 = np.asarray(inputs["W0"], dtype=np.float32)
    att_src0 = np.asarray(inputs["att_src0"], dtype=np.float32)
    att_dst0 = np.asarray(inputs["att_dst0"], dtype=np.float32)
    bias0 = np.asarray(inputs["bias0"], dtype=np.float32)
    W_res = np.asarray(inputs["W_res"], dtype=np.float32)
    att_src_res = np.asarray(inputs["att_src_res"], dtype=np.float32)
    att_dst_res = np.asarray(inputs["att_dst_res"], dtype=np.float32)
    bias_res = np.asarray(inputs["bias_res"], dtype=np.float32)
    gamma = np.asarray(inputs["gamma"], dtype=np.float32)
    beta = np.asarray(inputs["beta"], dtype=np.float32)
    W1p = np.asarray(inputs["W1p"], dtype=np.float32)
    W2p = np.asarray(inputs["W2p"], dtype=np.float32)

    prep = _host_prep(x, edge_index, W0, att_src0, att_dst0)
    phase = os.environ.get("GNN_PHASE", "full")
    D_key = (tuple(prep["D_sched"].tolist()), phase)
    if D_key not in _NC_CACHE:
        _NC_CACHE[D_key] = _build_nc(prep["D_sched"], prep["DTOT"], phase)
    nc = _NC_CACHE[D_key]

    # constants
    wcat = np.zeros((128, L, 132), dtype=np.float32)
    for i in range(L):
        wcat[:, i, 0:HID] = W_res[i]
        wcat[:, i, 128] = W_res[i] @ att_src_res[i]
        wcat[:, i, 129] = W_res[i] @ att_dst_res[i]
    gam = np.ascontiguousarray(gamma.T)        # [128, L]
    bet = np.ascontiguousarray(beta.T)
    biasr = np.zeros((128, L + 1, HID), dtype=np.float32)
    biasr[:, 0, :] = bias0[None, :]
    for i in range(L):
        biasr[:, i + 1, :] = bias_res[i][None, :]
    g0rep = np.broadcast_to(gamma[0][None, :], (128, HID)).copy()
    b0rep = np.broadcast_to(beta[0][None, :], (128, HID)).copy()
    idf = np.eye(128, dtype=np.float32)
    idb = np.eye(128, dtype=np.float32).astype(BF)

    in_maps = []
    for c in range(NCORE):
        w1p_c = np.concatenate([W1p[8 * c + g] for g in range(8)], axis=1)
        w2p_c = np.concatenate([W2p[8 * c + g] for g in range(8)], axis=1)
        in_maps.append({
            "g0": prep["g0"], "idx": prep["idx"][c], "mask": prep["mask"][c],
            "ad0": prep["a_d0"][c], "invl": prep["inv_local"][c],
            "wcat": wcat, "gam": gam, "bet": bet, "biasr": biasr,
            "g0rep": g0rep, "b0rep": b0rep, "idf": idf, "idb": idb,
            "w1p": np.ascontiguousarray(w1p_c),
            "w2p": np.ascontiguousarray(w2p_c),
        })

    trace = os.environ.get("GNN_TRACE", "0") == "1"
    res = run_bass_kernel_spmd(nc, in_maps, core_ids=list(range(NCORE)),
                               trace=trace)
    if trace:
        print("HW exec time:", res.exec_time_ns, "ns")
        print("trace:", res.instructions_and_trace[1]
              if res.instructions_and_trace else None)

    if os.environ.get("GNN_PHASE", "full") != "full":
        kernel.debug_h = np.stack(
            [res.results[c]["outh"] for c in range(NCORE)])
        kernel.debug_prep = prep
    out = np.concatenate([res.results[c]["out"] for c in range(NCORE)], axis=0)
    return out.reshape(NN, BS, OUT).astype(np.float32)


if __name__ == "__main__":
    pass
